# revision 12
# baseline (speedup 1.0000x reference)
"""Masked mean-pool (NonZeroAvgPool) Trainium2 Bass kernel.

out[b, d] = sum_s (tokens[b,s] != 0) * x[b,s,d] / sum_s (tokens[b,s] != 0)

Full shapes: x [16, 4096, 512] f32, tokens [16, 4096] i32 -> out [16, 512] f32.
Sharding: pure data parallel over batch; 2 batches per core on 8 cores.

Per-core program (shapes [2, 4096, 512] / [2, 4096] -> [2, 512]):
  - sequence rows are indexed s = p*32 + c  (p: SBUF partition, c: chunk)
    so every DMA is contiguous per partition.
  - valid[p, c] = (tokens != 0) as f32 via DVE not_equal
  - count      = ones[128,1].T @ rowsum(valid)        (PE, [1,1] PSUM)
  - num[1, D]  = sum_c valid[:, c].T @ x_tile[:, c, :] (PE, accumulated in PSUM)
  - out row    = num * (1/count)                       (DVE)
"""

from contextlib import ExitStack

import numpy as np

import concourse.bacc as bacc
import concourse.bass as bass
import concourse.tile as tile
from concourse import mybir
from concourse.bass_utils import run_bass_kernel_spmd

B, S, D = 16, 4096, 512
NCORES = 8
BPC = B // NCORES  # batches per core = 2
P = 128            # SBUF partitions
CPB = S // P       # chunks per batch = 32
import os

# DMA budget: this toolchain lowers every DMA to a static descriptor that can
# encode only ONE sem wait, and there are only 8 DMAHW completion lanes. More
# than 8 DMAs per core reuses a lane, which adds a second (lane-ordering) wait
# and fails walrus codegen. So: 1 tok DMA + 4 x DMAs + 1 out DMA = 6 total.
GRP = int(os.environ.get("K_GRP", "16"))  # chunks per x DMA group
NGRP = CPB // GRP  # x DMA groups per batch
X_ENGINE = os.environ.get("K_XENG", "sync")  # sync | act | gpsimd
# One private slot per x DMA (slot reuse would also add a WAW wait).
XBUFS = int(os.environ.get("K_XBUFS", str(NGRP * BPC)))

_NC = None


def _build_nc():
    # Bacc (not plain Bass): its compile() runs generate_event_semaphores,
    # which splits multi-wait instructions onto InstEventSemaphore — TRN2
    # instructions can carry at most one sem wait.
    nc = bacc.Bacc(trn_type="TRN2")
    x = nc.dram_tensor("x", [BPC, S, D], mybir.dt.float32, kind="ExternalInput")
    tokens = nc.dram_tensor("tokens", [BPC, S], mybir.dt.int32, kind="ExternalInput")
    out = nc.dram_tensor("out", [BPC, D], mybir.dt.float32, kind="ExternalOutput")

    # s = p*CPB + c : per-partition contiguous rows
    xa = x[:].rearrange("b (p c) d -> b p c d", p=P)   # [BPC, 128, 32, 512]
    ta = tokens[:].rearrange("b (p c) -> p b c", p=P)  # [128, BPC, 32]
    oa = out[:].rearrange("b d -> (b d)")              # [BPC*512]

    with TileKernel(nc) as tk:
        tk.body(xa, ta, oa)
    nc.compile()
    return nc


class TileKernel:
    def __init__(self, nc):
        self.nc = nc
        self.ctx = ExitStack()
        self.tc = None

    def __enter__(self):
        self.tc = self.ctx.enter_context(tile.TileContext(self.nc))
        return self

    def __exit__(self, *exc):
        return self.ctx.__exit__(*exc)

    def body(self, xa, ta, oa):
        nc = self.nc
        tc = self.tc
        ctx = self.ctx

        xpool = ctx.enter_context(tc.tile_pool(name="xpool", bufs=XBUFS))
        vpool = ctx.enter_context(tc.tile_pool(name="vpool", bufs=2))
        spool = ctx.enter_context(tc.tile_pool(name="spool", bufs=2))
        singles = ctx.enter_context(tc.tile_pool(name="singles", bufs=1))
        psum = ctx.enter_context(tc.tile_pool(name="psum", bufs=2, space="PSUM"))

        ones = singles.tile([P, 1], mybir.dt.float32)
        nc.vector.memset(ones, 1.0)
        xeng = {"sync": nc.sync, "act": nc.scalar, "gpsimd": nc.gpsimd}[X_ENGINE]

        # --- mask + counts for both batches (one tok DMA) --------------------
        tok = vpool.tile([P, BPC, CPB], mybir.dt.int32)
        nc.sync.dma_start(out=tok, in_=ta)
        valid = vpool.tile([P, BPC, CPB], mybir.dt.float32)
        nc.vector.tensor_scalar(
            out=valid, in0=tok, scalar1=0, scalar2=None,
            op0=mybir.AluOpType.not_equal,
        )
        rowsum = spool.tile([P, BPC], mybir.dt.float32)
        nc.vector.reduce_sum(out=rowsum, in_=valid, axis=mybir.AxisListType.X)

        obig = spool.tile([1, BPC * D], mybir.dt.float32)

        for b in range(BPC):
            cnt = psum.tile([1, 1], mybir.dt.float32)
            nc.tensor.matmul(cnt, ones, rowsum[:, b:b + 1], start=True, stop=True)
            recip = spool.tile([1, 1], mybir.dt.float32)
            nc.vector.reciprocal(recip, cnt)

            # --- masked sum ---------------------------------------------------
            num = psum.tile([1, D], mybir.dt.float32)
            for g in range(NGRP):
                xt = xpool.tile([P, GRP, D], mybir.dt.float32)
                xeng.dma_start(out=xt, in_=xa[b, :, g * GRP:(g + 1) * GRP, :])
                for k in range(GRP):
                    c = g * GRP + k
                    nc.tensor.matmul(
                        num, valid[:, b, c:c + 1], xt[:, k, :],
                        start=(c == 0), stop=(c == CPB - 1),
                    )

            # --- divide -------------------------------------------------------
            nc.vector.tensor_scalar_mul(obig[:, b * D:(b + 1) * D], num, recip)

        # --- single store of both output rows --------------------------------
        nc.sync.dma_start(out=oa, in_=obig)


def _get_nc():
    global _NC
    if _NC is None:
        _NC = _build_nc()
    return _NC


def _shard(x, tokens):
    x = np.ascontiguousarray(np.asarray(x, dtype=np.float32))
    tokens = np.ascontiguousarray(np.asarray(tokens, dtype=np.int32))
    return [
        {
            "x": x[c * BPC:(c + 1) * BPC],
            "tokens": tokens[c * BPC:(c + 1) * BPC],
        }
        for c in range(NCORES)
    ]


def kernel(x, tokens):
    res = run_bass_kernel_spmd(_get_nc(), _shard(x, tokens), core_ids=list(range(NCORES)))
    return np.concatenate([r["out"] for r in res.results], axis=0)


def _install_ntff_shim():
    """The agent image's antenv lacks axon_hooks, so bass_utils' trace path
    can't find the NTFF hook. Recreate the tiny get/set module and register
    trn_boot's ctypes-based hook against the injected libaxon_pjrt.so."""
    import sys
    import types

    if "antenv.axon_hooks" in sys.modules:
        return
    mod = types.ModuleType("antenv.axon_hooks")
    state = {"hook": None}
    mod.set_axon_ntff_profile_hook = lambda h: state.__setitem__("hook", h)
    mod.get_axon_ntff_profile_hook = lambda: state["hook"]
    sys.modules["antenv.axon_hooks"] = mod
    try:
        from trn_agent_boot.trn_boot import _ntff_profile_via_ctypes

        mod.set_axon_ntff_profile_hook(
            _ntff_profile_via_ctypes("/opt/axon/libaxon_pjrt.so")
        )
    except Exception:
        pass


def kernel_profiled(x, tokens):
    """Same as kernel() but with NTFF tracing; returns (out, BassKernelResults)."""
    _install_ntff_shim()
    res = run_bass_kernel_spmd(
        _get_nc(), _shard(x, tokens), core_ids=list(range(NCORES)), trace=True
    )
    out = np.concatenate([r["out"] for r in res.results], axis=0)
    return out, res


# revision 15
# speedup vs baseline: 1.3673x; 1.3673x over previous
"""Masked mean-pool (NonZeroAvgPool) Trainium2 Bass kernel.

out[b, d] = sum_s (tokens[b,s] != 0) * x[b,s,d] / sum_s (tokens[b,s] != 0)

Full shapes: x [16, 4096, 512] f32, tokens [16, 4096] i32 -> out [16, 512] f32.
Sharding: pure data parallel over batch; 2 batches per core on 8 cores.

Per-core program (shapes [2, 4096, 512] / [2, 4096] -> [2, 512]):
  - sequence rows are indexed s = p*32 + c  (p: SBUF partition, c: chunk)
    so every DMA is contiguous per partition.
  - valid[p, c] = (tokens != 0) as f32 via DVE not_equal
  - count      = ones[128,1].T @ rowsum(valid)        (PE, [1,1] PSUM)
  - num[1, D]  = sum_c valid[:, c].T @ x_tile[:, c, :] (PE, accumulated in PSUM)
  - out row    = num * (1/count)                       (DVE)
"""

from contextlib import ExitStack

import numpy as np

import concourse.bacc as bacc
import concourse.bass as bass
import concourse.tile as tile
from concourse import mybir
from concourse.bass_utils import run_bass_kernel_spmd

B, S, D = 16, 4096, 512
NCORES = 8
BPC = B // NCORES  # batches per core = 2
P = 128            # SBUF partitions
CPB = S // P       # chunks per batch = 32
import os

# DMA budget: this toolchain lowers every DMA to a static descriptor that can
# encode only ONE sem wait, and there are only 8 DMAHW completion lanes. More
# than 8 DMAs per core reuses a lane, which adds a second (lane-ordering) wait
# and fails walrus codegen. So: 1 tok DMA + 4 x DMAs + 1 out DMA = 6 total.
GRP = int(os.environ.get("K_GRP", "16"))  # chunks per x DMA group
NGRP = CPB // GRP  # x DMA groups per batch
X_ENGINE = os.environ.get("K_XENG", "sync")  # sync | act | gpsimd
# One private slot per x DMA (slot reuse would also add a WAW wait).
XBUFS = int(os.environ.get("K_XBUFS", str(NGRP * BPC)))

_NC = None


def _build_nc():
    # Bacc (not plain Bass): its compile() runs generate_event_semaphores,
    # which splits multi-wait instructions onto InstEventSemaphore — TRN2
    # instructions can carry at most one sem wait.
    nc = bacc.Bacc(trn_type="TRN2")
    x = nc.dram_tensor("x", [BPC, S, D], mybir.dt.float32, kind="ExternalInput")
    tokens = nc.dram_tensor("tokens", [BPC, S], mybir.dt.int32, kind="ExternalInput")
    out = nc.dram_tensor("out", [BPC, D], mybir.dt.float32, kind="ExternalOutput")

    # s = p*CPB + c : per-partition contiguous rows
    xa = x[:].rearrange("b (p c) d -> b p c d", p=P)   # [BPC, 128, 32, 512]
    ta = tokens[:].rearrange("b (p c) -> p b c", p=P)  # [128, BPC, 32]
    oa = out[:].rearrange("b d -> (b d)")              # [BPC*512]

    with TileKernel(nc) as tk:
        tk.body(xa, ta, oa)
    nc.compile()
    return nc


class TileKernel:
    def __init__(self, nc):
        self.nc = nc
        self.ctx = ExitStack()
        self.tc = None

    def __enter__(self):
        self.tc = self.ctx.enter_context(tile.TileContext(self.nc))
        return self

    def __exit__(self, *exc):
        return self.ctx.__exit__(*exc)

    def body(self, xa, ta, oa):
        nc = self.nc
        tc = self.tc
        ctx = self.ctx

        xpool = ctx.enter_context(tc.tile_pool(name="xpool", bufs=XBUFS))
        vpool = ctx.enter_context(tc.tile_pool(name="vpool", bufs=2))
        spool = ctx.enter_context(tc.tile_pool(name="spool", bufs=2))
        singles = ctx.enter_context(tc.tile_pool(name="singles", bufs=1))
        psum = ctx.enter_context(tc.tile_pool(name="psum", bufs=2, space="PSUM"))

        ones = singles.tile([P, 1], mybir.dt.float32)
        nc.vector.memset(ones, 1.0)
        xeng = {"sync": nc.sync, "act": nc.scalar, "gpsimd": nc.gpsimd}[X_ENGINE]

        # --- mask + counts for both batches (one tok DMA) --------------------
        tok = vpool.tile([P, BPC, CPB], mybir.dt.int32)
        nc.sync.dma_start(out=tok, in_=ta)
        # valid is declared float32r so the fp32r matmul's verifier sees a
        # rounded producer; its values (0.0/1.0) are exact in any precision.
        valid = vpool.tile([P, BPC, CPB], mybir.dt.float32r)
        nc.vector.tensor_scalar(
            out=valid, in0=tok, scalar1=0, scalar2=None,
            op0=mybir.AluOpType.not_equal,
        )
        rowsum = spool.tile([P, BPC], mybir.dt.float32)
        nc.vector.reduce_sum(
            out=rowsum, in_=valid.bitcast(mybir.dt.float32),
            axis=mybir.AxisListType.X,
        )

        obig = spool.tile([1, BPC * D], mybir.dt.float32)

        for b in range(BPC):
            cnt = psum.tile([1, 1], mybir.dt.float32)
            nc.tensor.matmul(cnt, ones, rowsum[:, b:b + 1], start=True, stop=True)
            recip = spool.tile([1, 1], mybir.dt.float32)
            nc.vector.reciprocal(recip, cnt)

            # --- masked sum ---------------------------------------------------
            num = psum.tile([1, D], mybir.dt.float32)
            for g in range(NGRP):
                # float32r: single-pass fp32 matmul (4x faster than fp32's two
                # half-rate passes). Same 4-byte layout as fp32 so the DMA is a
                # pure bit copy; the PE truncates low mantissa bits, mask
                # weights are exact 0/1, and PSUM still accumulates in fp32.
                xt = xpool.tile([P, GRP, D], mybir.dt.float32r)
                xeng.dma_start(out=xt, in_=xa[b, :, g * GRP:(g + 1) * GRP, :].bitcast(mybir.dt.float32r))
                for k in range(GRP):
                    c = g * GRP + k
                    nc.tensor.matmul(
                        num, valid[:, b, c:c + 1], xt[:, k, :],
                        start=(c == 0), stop=(c == CPB - 1),
                    )

            # --- divide -------------------------------------------------------
            nc.vector.tensor_scalar_mul(obig[:, b * D:(b + 1) * D], num, recip)

        # --- single store of both output rows --------------------------------
        nc.sync.dma_start(out=oa, in_=obig)


def _get_nc():
    global _NC
    if _NC is None:
        _NC = _build_nc()
    return _NC


def _shard(x, tokens):
    x = np.ascontiguousarray(np.asarray(x, dtype=np.float32))
    tokens = np.ascontiguousarray(np.asarray(tokens, dtype=np.int32))
    return [
        {
            "x": x[c * BPC:(c + 1) * BPC],
            "tokens": tokens[c * BPC:(c + 1) * BPC],
        }
        for c in range(NCORES)
    ]


def kernel(x, tokens):
    res = run_bass_kernel_spmd(_get_nc(), _shard(x, tokens), core_ids=list(range(NCORES)))
    return np.concatenate([r["out"] for r in res.results], axis=0)


def _install_ntff_shim():
    """The agent image's antenv lacks axon_hooks, so bass_utils' trace path
    can't find the NTFF hook. Recreate the tiny get/set module and register
    trn_boot's ctypes-based hook against the injected libaxon_pjrt.so."""
    import sys
    import types

    if "antenv.axon_hooks" in sys.modules:
        return
    mod = types.ModuleType("antenv.axon_hooks")
    state = {"hook": None}
    mod.set_axon_ntff_profile_hook = lambda h: state.__setitem__("hook", h)
    mod.get_axon_ntff_profile_hook = lambda: state["hook"]
    sys.modules["antenv.axon_hooks"] = mod
    try:
        from trn_agent_boot.trn_boot import _ntff_profile_via_ctypes

        mod.set_axon_ntff_profile_hook(
            _ntff_profile_via_ctypes("/opt/axon/libaxon_pjrt.so")
        )
    except Exception:
        pass


def kernel_profiled(x, tokens):
    """Same as kernel() but with NTFF tracing; returns (out, BassKernelResults)."""
    _install_ntff_shim()
    res = run_bass_kernel_spmd(
        _get_nc(), _shard(x, tokens), core_ids=list(range(NCORES)), trace=True
    )
    out = np.concatenate([r["out"] for r in res.results], axis=0)
    return out, res
